# revision 4
# baseline (speedup 1.0000x reference)
"""CBOW-subword embedding lookup on 8 TRN2 NeuronCores.

out[b, s, :] = W[seq[b,s]] + W[prefix[b,s]] + W[postfix[b,s]]

Sharding: data-parallel on the batch dim (16 sequences per core), weight
table replicated (stays in HBM; gathered via indirect DMA).

Per-core kernel: token t = p*128 + c lives on partition p, column c.
For each column c (128 tokens), three indirect-DMA gathers (one index per
partition — the HW vector-indirect primitive) fetch the 3 embedding rows;
the SDMA CCE accumulates them in SBUF (compute_op=add). Every GRP columns
the staging tile is stored contiguously to DRAM.
"""

import numpy as np

import concourse.bacc as bacc
import concourse.bass as bass
import concourse.mybir as mybir
import concourse.tile as tile
from concourse.bass_utils import run_bass_kernel_spmd

B, S, V, D = 128, 1024, 100000, 128
NCORES = 8
BSH = B // NCORES        # 16 sequences per core
TOK = BSH * S            # 16384 tokens per core
P = 128                  # SBUF partitions
COLS = TOK // P          # 128 tokens per partition
GRP = 16                 # columns accumulated per staging tile / store
NGRP = COLS // GRP       # 8 stores per core
STAGE_BUFS = 3

_CACHE: dict[str, object] = {}


def build_nc():
    nc = bacc.Bacc("TRN2", target_bir_lowering=False, debug=False)

    seq = nc.declare_dram_parameter("seq", [TOK], mybir.dt.int32, isOutput=False)
    pre = nc.declare_dram_parameter("pre", [TOK], mybir.dt.int32, isOutput=False)
    post = nc.declare_dram_parameter("post", [TOK], mybir.dt.int32, isOutput=False)
    wt = nc.declare_dram_parameter("wt", [V, D], mybir.dt.float32, isOutput=False)
    out = nc.declare_dram_parameter("out", [TOK, D], mybir.dt.float32, isOutput=True)

    seq_v = seq[:].rearrange("(p c) -> p c", p=P)
    pre_v = pre[:].rearrange("(p c) -> p c", p=P)
    post_v = post[:].rearrange("(p c) -> p c", p=P)
    out_v = out[:].rearrange("(p grp g) d -> grp p (g d)", p=P, grp=NGRP, g=GRP)

    with tile.TileContext(nc) as tc:
        with (
            tc.tile_pool(name="idx", bufs=1) as idx_pool,
            tc.tile_pool(name="stage", bufs=STAGE_BUFS) as stage_pool,
        ):
            idx_s = idx_pool.tile([P, COLS], mybir.dt.int32, tag="idx_s")
            idx_p = idx_pool.tile([P, COLS], mybir.dt.int32, tag="idx_p")
            idx_q = idx_pool.tile([P, COLS], mybir.dt.int32, tag="idx_q")
            nc.sync.dma_start(idx_s[:], seq_v)
            nc.sync.dma_start(idx_p[:], pre_v)
            nc.sync.dma_start(idx_q[:], post_v)

            for grp in range(NGRP):
                stage = stage_pool.tile([P, GRP * D], mybir.dt.float32, tag="stage")
                for g in range(GRP):
                    c = grp * GRP + g
                    dst = stage[:, g * D : (g + 1) * D]
                    for i, idx in enumerate((idx_s, idx_p, idx_q)):
                        nc.gpsimd.indirect_dma_start(
                            out=dst,
                            out_offset=None,
                            in_=wt[:],
                            in_offset=bass.IndirectOffsetOnAxis(
                                ap=idx[:, c : c + 1], axis=0
                            ),
                            compute_op=(
                                mybir.AluOpType.bypass
                                if i == 0
                                else mybir.AluOpType.add
                            ),
                        )
                nc.sync.dma_start(out_v[grp], stage[:])
    nc.compile()
    return nc


def _shard_inputs(sequence, prefix_idx, postfix_idx, weight):
    wt = np.ascontiguousarray(np.asarray(weight, dtype=np.float32))
    in_maps = []
    for c in range(NCORES):
        sl = slice(c * BSH, (c + 1) * BSH)
        in_maps.append(
            {
                "seq": np.ascontiguousarray(
                    np.asarray(sequence[sl], dtype=np.int32).reshape(TOK)
                ),
                "pre": np.ascontiguousarray(
                    np.asarray(prefix_idx[sl], dtype=np.int32).reshape(TOK)
                ),
                "post": np.ascontiguousarray(
                    np.asarray(postfix_idx[sl], dtype=np.int32).reshape(TOK)
                ),
                "wt": wt,
            }
        )
    return in_maps


def kernel(sequence, prefix_idx, postfix_idx, weight, **run_kwargs):
    if "nc" not in _CACHE:
        _CACHE["nc"] = build_nc()
    nc = _CACHE["nc"]
    in_maps = _shard_inputs(sequence, prefix_idx, postfix_idx, weight)
    res = run_bass_kernel_spmd(nc, in_maps, list(range(NCORES)), **run_kwargs)
    out = np.concatenate(
        [res.results[c]["out"].reshape(BSH, S, D) for c in range(NCORES)], axis=0
    )
    return out


# revision 6
# speedup vs baseline: 23.1479x; 23.1479x over previous
"""CBOW-subword embedding lookup on 8 TRN2 NeuronCores.

out[b, s, :] = W[seq[b,s]] + W[prefix[b,s]] + W[postfix[b,s]]

Sharding: data-parallel on the batch dim (16 sequences per core), weight
table replicated (stays in HBM; gathered via indirect DMA).

Per-core kernel: token t = p*128 + c lives on partition p, column c.
For each column c (128 tokens), three indirect-DMA gathers (one index per
partition — the HW vector-indirect primitive) fetch the 3 embedding rows;
the SDMA CCE accumulates them in SBUF (compute_op=add). Every GRP columns
the staging tile is stored contiguously to DRAM.
"""

import numpy as np

import concourse.bacc as bacc
import concourse.bass as bass
import concourse.mybir as mybir
import concourse.tile as tile
from concourse.bass_utils import run_bass_kernel_spmd

B, S, V, D = 128, 1024, 100000, 128
NCORES = 8
BSH = B // NCORES        # 16 sequences per core
TOK = BSH * S            # 16384 tokens per core
P = 128                  # SBUF partitions
COLS = TOK // P          # 128 tokens per partition
GRP = 16                 # columns accumulated per staging tile / store
NGRP = COLS // GRP       # 8 stores per core
STAGE_BUFS = 3

_CACHE: dict[str, object] = {}


def build_nc(repeat=1):
    nc = bacc.Bacc("TRN2", target_bir_lowering=False, debug=False)

    seq = nc.declare_dram_parameter("seq", [TOK], mybir.dt.int32, isOutput=False)
    pre = nc.declare_dram_parameter("pre", [TOK], mybir.dt.int32, isOutput=False)
    post = nc.declare_dram_parameter("post", [TOK], mybir.dt.int32, isOutput=False)
    wt = nc.declare_dram_parameter("wt", [V, D], mybir.dt.float32, isOutput=False)
    out = nc.declare_dram_parameter("out", [TOK, D], mybir.dt.float32, isOutput=True)

    seq_v = seq[:].rearrange("(p c) -> p c", p=P)
    pre_v = pre[:].rearrange("(p c) -> p c", p=P)
    post_v = post[:].rearrange("(p c) -> p c", p=P)
    out_v = out[:].rearrange("(p grp g) d -> grp p (g d)", p=P, grp=NGRP, g=GRP)

    with tile.TileContext(nc) as tc:
        with (
            tc.tile_pool(name="idx", bufs=1) as idx_pool,
            tc.tile_pool(name="stage", bufs=STAGE_BUFS) as stage_pool,
        ):
            idx_s = idx_pool.tile([P, COLS], mybir.dt.int32, tag="idx_s")
            idx_p = idx_pool.tile([P, COLS], mybir.dt.int32, tag="idx_p")
            idx_q = idx_pool.tile([P, COLS], mybir.dt.int32, tag="idx_q")
            nc.sync.dma_start(idx_s[:], seq_v)
            nc.sync.dma_start(idx_p[:], pre_v)
            nc.sync.dma_start(idx_q[:], post_v)

            for grp in range(NGRP * repeat):
                grp = grp % NGRP
                stage = stage_pool.tile([P, GRP * D], mybir.dt.float32, tag="stage")
                for g in range(GRP):
                    c = grp * GRP + g
                    dst = stage[:, g * D : (g + 1) * D]
                    for i, idx in enumerate((idx_s, idx_p, idx_q)):
                        nc.gpsimd.indirect_dma_start(
                            out=dst,
                            out_offset=None,
                            in_=wt[:],
                            in_offset=bass.IndirectOffsetOnAxis(
                                ap=idx[:, c : c + 1], axis=0
                            ),
                            compute_op=(
                                mybir.AluOpType.bypass
                                if i == 0
                                else mybir.AluOpType.add
                            ),
                        )
                nc.sync.dma_start(out_v[grp], stage[:])
    nc.compile()
    return nc


def _shard_inputs(sequence, prefix_idx, postfix_idx, weight):
    wt = np.ascontiguousarray(np.asarray(weight, dtype=np.float32))
    in_maps = []
    for c in range(NCORES):
        sl = slice(c * BSH, (c + 1) * BSH)
        in_maps.append(
            {
                "seq": np.ascontiguousarray(
                    np.asarray(sequence[sl], dtype=np.int32).reshape(TOK)
                ),
                "pre": np.ascontiguousarray(
                    np.asarray(prefix_idx[sl], dtype=np.int32).reshape(TOK)
                ),
                "post": np.ascontiguousarray(
                    np.asarray(postfix_idx[sl], dtype=np.int32).reshape(TOK)
                ),
                "wt": wt,
            }
        )
    return in_maps


def kernel(sequence, prefix_idx, postfix_idx, weight, **run_kwargs):
    if "nc" not in _CACHE:
        _CACHE["nc"] = build_nc()
    nc = _CACHE["nc"]
    in_maps = _shard_inputs(sequence, prefix_idx, postfix_idx, weight)
    res = run_bass_kernel_spmd(nc, in_maps, list(range(NCORES)), **run_kwargs)
    out = np.concatenate(
        [res.results[c]["out"].reshape(BSH, S, D) for c in range(NCORES)], axis=0
    )
    return out
